# revision 12
# baseline (speedup 1.0000x reference)
"""DeepSeek-V2-Lite-style MoE layer on 8 Trainium2 NeuronCores, v2.

Design vs v1 baseline:
- Data-derived per-expert capacities (host computes routing counts in numpy,
  pads +8 and rounds to 16, caps at C=256), experts assigned to cores one per
  global-size octile so every core runs the same "pattern" of capacities
  (sum ~848 slots/core instead of 2048) -> ~2.4x less expert GEMM work.
- All weights (w13/w2/shared) stored as float8e3 (E3M4) scaled by 64, used
  directly as one operand of bf16 matmuls (PE runs mixed fp8xbf16 at bf16
  rate) -> half the weight HBM traffic.
- Combine phase replaced by indirect scatter-add DMA (compute_op=add):
  down-proj outputs are scaled by the routing weight (scalar engine, per-slot
  scale) and scatter-added straight into a token-order accumulator in DRAM;
  the shared-MLP output is written there first. No gather combine, no
  full-size fp32 roundtrips.
- Routing builds a slot table via a "spread" layout [tokens, total_slots]:
  per-slot expert ids / in-expert positions are host constants; position
  cumsum, token table and per-slot weights are built with small fp16 matmuls.
- ReduceScatter in bf16 over the accumulator; per-core output rows 128c..

Config (DeepSeek-V2-Lite): T=1024 H=2048 E=64 K=6 I=1024 G=8 TG=3 C=256
"""
import numpy as np
import ml_dtypes
from contextlib import ExitStack

import concourse.bass as bass
import concourse.tile as tile
from concourse import bacc
from concourse import mybir
from concourse import bass_utils

T, H, E, K, I, G, TG, C = 1024, 2048, 64, 6, 1024, 8, 3, 256
S = 2048                  # full shared intermediate
RSF = 2.5
NCORES = 8
SL = S // NCORES          # shared-intermediate slice per core (TP)
P = 128
NT = T // P               # token tiles
NKH = H // P              # K-tiles over H
NIT = I // P              # I-tiles
BIG = 1.0e30
WS = 64.0                 # fp8 weight pre-scale
DS = 1.0 / (WS * WS * WS)  # descale after gate_up*up*down chain (2^-18)

F32 = mybir.dt.float32
F16 = mybir.dt.float16
U32 = mybir.dt.uint32
BF16 = mybir.dt.bfloat16
F8E3 = mybir.dt.float8e3
U8 = mybir.dt.uint8
AX = mybir.AxisListType
ALU = mybir.AluOpType
ACTF = mybir.ActivationFunctionType

bf16 = ml_dtypes.bfloat16
f8e3 = ml_dtypes.float8_e3m4


# ---------------------------------------------------------------- host routing
def _host_topk_ids(hs, gw, gb):
    """numpy replica of reference._grouped_topk selection (ids only)."""
    logits = hs.astype(np.float64) @ gw.T.astype(np.float64)
    scores = 1.0 / (1.0 + np.exp(-logits))
    sc = scores + gb.astype(np.float64)
    grp = sc.reshape(T, G, E // G)
    g2 = np.sort(grp, axis=-1)[:, :, -2:].sum(-1)
    top_g = np.argsort(-g2, kind="stable", axis=1)[:, :TG]
    gmask = np.zeros((T, G))
    np.put_along_axis(gmask, top_g, 1.0, axis=1)
    masked = np.where(gmask[:, :, None] > 0, grp, -np.inf).reshape(T, E)
    return np.argsort(-masked, kind="stable", axis=1)[:, :K]


def _plan(cnt):
    """Caps, octile expert->core assignment, shared capacity pattern."""
    caps = np.where(cnt == 0, 0,
                    np.minimum(C, ((cnt + 8 + 15) // 16) * 16)).astype(int)
    order = np.argsort(-caps, kind="stable")
    pattern, experts_at = [], []   # experts_at[j][c] = expert id
    for j in range(8):
        octile = order[8 * j:8 * j + 8]
        pattern.append(int(caps[octile].max()))
        experts_at.append([int(octile[c]) for c in range(NCORES)])
    keep = [j for j in range(8) if pattern[j] > 0]
    pattern = [pattern[j] for j in keep]
    experts_at = [experts_at[j] for j in keep]
    offs = np.concatenate([[0], np.cumsum(pattern)]).astype(int)
    return pattern, experts_at, offs, int(offs[-1])


# ---------------------------------------------------------------- device build
def build_program(pattern, offs, SS):
    NJ = len(pattern)
    nc = bacc.Bacc("TRN2", target_bir_lowering=False, debug=False,
                   num_devices=NCORES)

    hs_pad = nc.dram_tensor("hs_pad", [T + 1, H], BF16, kind="ExternalInput")
    hsT = nc.dram_tensor("hsT", [H, T], F32, kind="ExternalInput")
    hsT_bf = nc.dram_tensor("hsT_bf", [H, T], BF16, kind="ExternalInput")
    gwT = nc.dram_tensor("gwT", [H, E], F32, kind="ExternalInput")
    bias_bc = nc.dram_tensor("bias_bc", [P, E], F32, kind="ExternalInput")
    u128 = nc.dram_tensor("u128", [P, P], F16, kind="ExternalInput")
    ones128 = nc.dram_tensor("ones128", [P, P], F16, kind="ExternalInput")
    onesc = nc.dram_tensor("onesc", [P, 1], F16, kind="ExternalInput")
    tokcol = nc.dram_tensor("tokcol", [P, NT], F16, kind="ExternalInput")
    ident_b = nc.dram_tensor("ident_b", [P, P], BF16, kind="ExternalInput")
    eslot = nc.dram_tensor("eslot", [P, SS], F16, kind="ExternalInput")
    ciota = nc.dram_tensor("ciota", [P, SS], F16, kind="ExternalInput")
    w13_pk = nc.dram_tensor("w13_pk", [NJ, P, NIT * NKH * 256], F8E3,
                            kind="ExternalInput")
    w2_pk = nc.dram_tensor("w2_pk", [NJ, 4, P, NIT * 512], F8E3,
                            kind="ExternalInput")
    sgu_loc = nc.dram_tensor("sgu_loc", [H, 2 * SL], BF16, kind="ExternalInput")
    sd_loc = nc.dram_tensor("sd_loc", [SL, H], BF16, kind="ExternalInput")

    table = nc.dram_tensor("table", [SS, 1], U32, kind="Internal")
    wtbl = nc.dram_tensor("wtbl", [SS, 1], F32, kind="Internal")
    acc = nc.dram_tensor("acc", [T + 1, H], BF16, kind="Internal")
    cc_out = nc.dram_tensor("cc_out", [P, H], BF16, kind="Internal")
    out_t = nc.dram_tensor("out", [P, H], BF16, kind="ExternalOutput")

    ch_bounds = list(range(0, SS, 512)) + [SS]
    chunks = [(ch_bounds[i], ch_bounds[i + 1]) for i in range(len(ch_bounds) - 1)]

    with tile.TileContext(nc) as tc, ExitStack() as ctx:
        cst = ctx.enter_context(tc.tile_pool(name="cst", bufs=1))
        rt = ctx.enter_context(tc.tile_pool(name="rt", bufs=2))
        sc_pool = ctx.enter_context(tc.tile_pool(name="scp", bufs=8))
        sp_ctx = ExitStack()
        sp = sp_ctx.enter_context(tc.tile_pool(name="sp", bufs=8))
        tkp = sp_ctx.enter_context(tc.tile_pool(name="tkp", bufs=4))

        # ---------------- constants
        bias_t = cst.tile([P, E], F32, tag="bias")
        nc.sync.dma_start(bias_t[:], bias_bc[:])
        u_t = cst.tile([P, P], F16, tag="u128")
        nc.sync.dma_start(u_t[:], u128[:])
        o128_t = cst.tile([P, P], F16, tag="o128")
        nc.sync.dma_start(o128_t[:], ones128[:])
        onesc_t = cst.tile([P, 1], F16, tag="onesc")
        nc.sync.dma_start(onesc_t[:], onesc[:])
        tokcol_t = cst.tile([P, NT], F16, tag="tokcol")
        nc.sync.dma_start(tokcol_t[:], tokcol[:])
        id_t = cst.tile([P, P], BF16, tag="ident")
        nc.sync.dma_start(id_t[:], ident_b[:])
        eslot_t = cst.tile([P, SS], F16, tag="eslot")
        nc.sync.dma_start(eslot_t[:], eslot[:])
        ciota_t = cst.tile([P, SS], F16, tag="ciota")
        nc.sync.dma_start(ciota_t[:], ciota[:])
        gw_t = cst.tile([P, NKH * E], F32, tag="gwT")
        nc.sync.dma_start(gw_t[:].rearrange("p (kt e) -> p kt e", kt=NKH),
                          gwT[:].rearrange("(kt p) e -> p kt e", p=P))
        iot_e = cst.tile([P, E], U32, tag="iote")
        nc.gpsimd.iota(iot_e[:], pattern=[[1, E]], base=0, channel_multiplier=0)
        acum_t = cst.tile([P, SS], F16, tag="acum")
        nc.vector.memset(acum_t[:], 0.0)

        # ============ phase R1: router matmuls + sigmoids
        scores_l = []
        with ExitStack() as lg_ctx:
            lg_pool = lg_ctx.enter_context(
                tc.tile_pool(name="lgps", bufs=NT, space="PSUM"))
            hst_pool = lg_ctx.enter_context(tc.tile_pool(name="hst", bufs=4))
            lgs = []
            for tt in range(NT):
                lg = lg_pool.tile([P, E], F32, tag="lg")
                lgs.append(lg)
            for kt in range(NKH):
                hv = hst_pool.tile([P, T], F32, tag="hstr")
                nc.sync.dma_start(hv[:], hsT[kt * P:(kt + 1) * P, :])
                for tt in range(NT):
                    nc.tensor.matmul(lgs[tt][:],
                                     hv[:, tt * P:(tt + 1) * P],
                                     gw_t[:, kt * E:(kt + 1) * E],
                                     start=(kt == 0), stop=(kt == NKH - 1))
            for tt in range(NT):
                scr = sc_pool.tile([P, E], F32, tag="scores")
                nc.scalar.activation(scr[:], lgs[tt][:], ACTF.Sigmoid)
                scores_l.append(scr)

        # ============ phase S: shared MLP (TP slice), writes acc rows 1..T
        with ExitStack() as s_ctx:
            sh_pool = s_ctx.enter_context(tc.tile_pool(name="sh", bufs=1))
            sd_pool = s_ctx.enter_context(tc.tile_pool(name="sd", bufs=1))
            hv2_pool = s_ctx.enter_context(tc.tile_pool(name="hv2", bufs=1))
            ash_pool = s_ctx.enter_context(tc.tile_pool(name="ash", bufs=2))
            ysb_pool = s_ctx.enter_context(tc.tile_pool(name="ysb", bufs=3))
            st_pool = s_ctx.enter_context(tc.tile_pool(name="st", bufs=2))
            hsh_ps_pool = s_ctx.enter_context(
                tc.tile_pool(name="hshps", bufs=1, space="PSUM"))
            ysh_ps_pool = s_ctx.enter_context(
                tc.tile_pool(name="yshps", bufs=2, space="PSUM"))
            sgu_all = sh_pool.tile([P, NKH * 2 * SL], BF16, tag="sguall")
            sgu_a3 = sgu_all[:].rearrange("p (kt c) -> p kt c", kt=NKH)
            hv_all = hv2_pool.tile([P, NKH * T], BF16, tag="hvall")
            hv_a3 = hv_all[:].rearrange("p (kt t) -> p kt t", kt=NKH)
            sd_all = sd_pool.tile([P, 2 * H], BF16, tag="sdall")
            sd_a3 = sd_all[:].rearrange("p (kt h) -> p kt h", kt=2)
            for q in range(4):
                nc.sync.dma_start(
                    hv_a3[:, q * 4:(q + 1) * 4, :],
                    hsT_bf[q * 4 * P:(q + 1) * 4 * P, :]
                    .rearrange("(kt p) t -> p kt t", p=P))
                nc.sync.dma_start(
                    sgu_a3[:, q * 4:(q + 1) * 4, :],
                    sgu_loc[q * 4 * P:(q + 1) * 4 * P, :]
                    .rearrange("(kt p) c -> p kt c", p=P))
            nc.sync.dma_start(
                sd_a3, sd_loc[:].rearrange("(kt p) h -> p kt h", p=P))
            for tcn in range(2):  # halves of T
                hsh_ps = hsh_ps_pool.tile([P, 4 * 512], F32, tag="hsh")
                for kt in range(NKH):
                    for mt in range(4):
                        nc.tensor.matmul(
                            hsh_ps[:, mt * 512:(mt + 1) * 512],
                            sgu_a3[:, kt, mt * P:(mt + 1) * P],
                            hv_a3[:, kt, tcn * 512:(tcn + 1) * 512],
                            start=(kt == 0), stop=(kt == NKH - 1))
                # silu(gate)*up on scalar+gpsimd (keep DVE free for routing)
                ash_t = ash_pool.tile([P, 2 * 512], BF16, tag="ash")
                for mt in range(2):
                    sil = st_pool.tile([P, 512], F32, tag="sil")
                    nc.scalar.activation(sil[:],
                                         hsh_ps[:, mt * 512:(mt + 1) * 512],
                                         ACTF.Sigmoid)
                    hg = st_pool.tile([P, 512], F32, tag="hg")
                    nc.scalar.activation(hg[:],
                                         hsh_ps[:, mt * 512:(mt + 1) * 512],
                                         ACTF.Copy)
                    hu = st_pool.tile([P, 512], F32, tag="hu")
                    nc.scalar.activation(hu[:],
                                         hsh_ps[:, (mt + 2) * 512:(mt + 3) * 512],
                                         ACTF.Copy)
                    nc.gpsimd.tensor_tensor(sil[:], sil[:], hg[:], ALU.mult)
                    nc.gpsimd.tensor_tensor(
                        ash_t[:, mt * 512:(mt + 1) * 512], sil[:],
                        hu[:], ALU.mult)
                for ts in range(4):
                    tglob = tcn * 4 + ts
                    for hc in range(4):
                        ysh_ps = ysh_ps_pool.tile([P, 512], F32, tag="ysh")
                        for kt in range(2):
                            nc.tensor.matmul(
                                ysh_ps[:],
                                ash_t[:, kt * 512 + ts * P:
                                      kt * 512 + (ts + 1) * P],
                                sd_a3[:, kt, hc * 512:(hc + 1) * 512],
                                start=(kt == 0), stop=(kt == 1))
                        ysb = ysb_pool.tile([P, 512], BF16, tag="ysb_sh")
                        nc.scalar.activation(ysb[:], ysh_ps[:], ACTF.Copy)
                        nc.sync.dma_start(
                            acc[1 + tglob * P:1 + (tglob + 1) * P,
                                hc * 512:(hc + 1) * 512], ysb[:])

        # ============ phase R2: grouped top-k + spread (DVE pass 1)
        wsp_l, asp_l, i8_l = [], [], []
        for it in range(NT):
            scores = scores_l[it]
            sc = rt.tile([P, E], F32, tag="sc")
            nc.vector.tensor_tensor(sc[:], scores[:], bias_t[:], ALU.add)
            grp = sc[:].rearrange("p (g e) -> p g e", g=G)
            m1 = rt.tile([P, G], F32, tag="m1")
            nc.vector.tensor_reduce(m1[:], grp, axis=AX.X, op=ALU.max)
            eq = rt.tile([P, E], F32, tag="eq")
            m1b = m1[:].rearrange("p (g o) -> p g o", o=1).broadcast_to((P, G, G))
            nc.vector.tensor_tensor(eq[:].rearrange("p (g e) -> p g e", g=G),
                                    grp, m1b, ALU.is_ge)
            pen = rt.tile([P, E], F32, tag="pen")
            nc.vector.tensor_scalar(pen[:], eq[:], -BIG, None, op0=ALU.mult)
            msk2 = rt.tile([P, E], F32, tag="msk2")
            nc.vector.tensor_tensor(msk2[:], sc[:], pen[:], ALU.add)
            m2 = rt.tile([P, G], F32, tag="m2")
            nc.vector.tensor_reduce(
                m2[:], msk2[:].rearrange("p (g e) -> p g e", g=G),
                axis=AX.X, op=ALU.max)
            g2 = rt.tile([P, G], F32, tag="g2")
            nc.vector.tensor_tensor(g2[:], m1[:], m2[:], ALU.add)

            gv8 = rt.tile([P, 8], F32, tag="gv8")
            gi8 = rt.tile([P, 8], U32, tag="gi8")
            nc.vector.max_with_indices(gv8[:], gi8[:], g2[:])
            gmask = rt.tile([P, G], F32, tag="gmask")
            nc.vector.tensor_tensor(gmask[:], g2[:],
                                    gv8[:, TG - 1:TG].broadcast_to((P, G)),
                                    ALU.is_ge)
            gm64 = rt.tile([P, E], U8, tag="gm64")
            gmb = gmask[:].rearrange("p (g o) -> p g o", o=1) \
                .broadcast_to((P, G, G))
            nc.vector.tensor_copy(gm64[:].rearrange("p (g e) -> p g e", g=G),
                                  gmb)
            scm = rt.tile([P, E], F32, tag="scm")
            nc.vector.memset(scm[:], -BIG)
            nc.vector.copy_predicated(scm[:], gm64[:], sc[:])
            v8 = rt.tile([P, 8], F32, tag="v8")
            i8 = sc_pool.tile([P, 8], U32, tag="i8")
            nc.vector.max_with_indices(v8[:], i8[:], scm[:])
            i8_l.append(i8)

            # per-k selected scores -> normalized weights
            scok = rt.tile([P, K], F32, tag="scok")
            for k in range(K):
                oh = rt.tile([P, E], F32, tag="oh")
                nc.vector.tensor_tensor(oh[:], iot_e[:],
                                        i8[:, k:k + 1].broadcast_to((P, E)),
                                        ALU.is_equal)
                tmp = rt.tile([P, E], F32, tag="ttmp")
                nc.vector.tensor_tensor(tmp[:], scores[:], oh[:], ALU.mult)
                nc.vector.tensor_reduce(scok[:, k:k + 1], tmp[:], axis=AX.X,
                                        op=ALU.add)
            ssum = rt.tile([P, 1], F32, tag="ssum")
            nc.vector.tensor_reduce(ssum[:], scok[:], axis=AX.X, op=ALU.add)
            nc.vector.tensor_scalar(ssum[:], ssum[:], 1e-20, None, op0=ALU.add)
            sinv = rt.tile([P, 1], F32, tag="sinv")
            nc.vector.reciprocal(sinv[:], ssum[:])
            nc.vector.tensor_scalar(sinv[:], sinv[:], RSF * DS, None,
                                    op0=ALU.mult)
            sckn = rt.tile([P, K], F32, tag="sckn")
            nc.vector.tensor_scalar(sckn[:], scok[:], sinv[:], None,
                                    op0=ALU.mult)
            i8f = rt.tile([P, 8], F16, tag="i8f")
            nc.vector.tensor_copy(i8f[:], i8[:])

            # spread per-slot weights: w_sp[t,s] = sum_k [eslot==e_k] * wk
            # 6 independent STTs on DVE; the accumulation adds on gpsimd
            w_sp = sp.tile([P, SS], F16, tag="wsp")
            for k in range(K):
                dst = w_sp if k == 0 else tkp.tile([P, SS], F16, tag="tk")
                nc.vector.scalar_tensor_tensor(
                    dst[:], eslot_t[:], i8f[:, k:k + 1],
                    sckn[:, k:k + 1].broadcast_to((P, SS)),
                    op0=ALU.is_equal, op1=ALU.mult)
                if k > 0:
                    nc.gpsimd.tensor_tensor(w_sp[:], w_sp[:], dst[:], ALU.add)
            a_sp = sp.tile([P, SS], F16, tag="asp")
            nc.vector.tensor_scalar(a_sp[:], w_sp[:], 0.0, None, op0=ALU.is_gt)
            wsp_l.append(w_sp)
            asp_l.append(a_sp)

        # ============ phase R3: cumsum + table build (tensor + DVE pass 2)
        with ExitStack() as r3_ctx:
            pos_ps_pool = r3_ctx.enter_context(
                tc.tile_pool(name="posps", bufs=2, space="PSUM"))
            aux_ps_pool = r3_ctx.enter_context(
                tc.tile_pool(name="auxps", bufs=1, space="PSUM"))
            aux_ps = aux_ps_pool.tile([33, SS], F32, tag="aux")
            for it in range(NT):
                a_sp, w_sp = asp_l[it], wsp_l[it]
                pos_ps = pos_ps_pool.tile([P, SS], F32, tag="pos")
                for c0, c1 in chunks:
                    nc.tensor.matmul(pos_ps[:, c0:c1], u_t[:],
                                     a_sp[:, c0:c1], start=True, stop=False)
                    nc.tensor.matmul(pos_ps[:, c0:c1], o128_t[:],
                                     acum_t[:, c0:c1], start=False, stop=True)
                pos16 = rt.tile([P, SS], F16, tag="pos16")
                nc.vector.tensor_copy(pos16[:], pos_ps[:])
                # running column-sum of selections (read by next tile's matmul)
                nc.vector.tensor_tensor(acum_t[:], acum_t[:], a_sp[:], ALU.add)
                eqm = rt.tile([P, SS], F16, tag="eqm")
                nc.vector.tensor_tensor(eqm[:], ciota_t[:], pos16[:],
                                        ALU.is_equal)
                pall = rt.tile([P, SS], F16, tag="pall")
                nc.vector.tensor_tensor(pall[:], eqm[:], a_sp[:], ALU.mult)
                pallw = rt.tile([P, SS], F16, tag="pallw")
                nc.vector.tensor_tensor(pallw[:], eqm[:], w_sp[:], ALU.mult)
                for c0, c1 in chunks:
                    nc.tensor.matmul(aux_ps[0:1, c0:c1],
                                     tokcol_t[:, it:it + 1], pall[:, c0:c1],
                                     start=(it == 0), stop=(it == NT - 1),
                                     tile_position=(0, 0))
                    nc.tensor.matmul(aux_ps[32:33, c0:c1],
                                     onesc_t[:], pallw[:, c0:c1],
                                     start=(it == 0), stop=(it == NT - 1),
                                     tile_position=(0, 32))

            tblf = rt.tile([1, SS], F32, tag="tblf")
            nc.vector.tensor_copy(tblf[:], aux_ps[0:1, :])
            tblu = rt.tile([1, SS], U32, tag="tblu")
            nc.vector.tensor_copy(tblu[:], tblf[:])
            nc.sync.dma_start(table[:], tblu[:])
            wtf = rt.tile([1, SS], F32, tag="wtf")
            nc.vector.tensor_copy(wtf[:], aux_ps[32:33, :])
            nc.sync.dma_start(wtbl[:], wtf[:])
        sp_ctx.close()

        # ============ phase E: expert GEMMs + scatter-add combine
        with ExitStack() as e_ctx:
            xe_pool = e_ctx.enter_context(tc.tile_pool(name="xe", bufs=3))
            xet_pool = e_ctx.enter_context(tc.tile_pool(name="xet", bufs=2))
            w13_pool = e_ctx.enter_context(tc.tile_pool(name="we", bufs=3))
            at_pool = e_ctx.enter_context(tc.tile_pool(name="at", bufs=2))
            w2_pool = e_ctx.enter_context(tc.tile_pool(name="w2", bufs=3))
            ys_pool = e_ctx.enter_context(tc.tile_pool(name="ys", bufs=4))
            idx_pool = e_ctx.enter_context(tc.tile_pool(name="idx", bufs=6))
            sil_pool = e_ctx.enter_context(tc.tile_pool(name="sile", bufs=3))
            tr_ps_pool = e_ctx.enter_context(
                tc.tile_pool(name="trps", bufs=2, space="PSUM"))
            ht_ps_pool = e_ctx.enter_context(
                tc.tile_pool(name="htps", bufs=4, space="PSUM"))
            y_ps_pool = e_ctx.enter_context(
                tc.tile_pool(name="yps", bufs=2, space="PSUM"))

            for j in range(NJ):
                capj, offj = pattern[j], int(offs[j])
                nct = (capj + P - 1) // P
                rws = [min(P, capj - ct * P) for ct in range(nct)]
                # gather + transpose to [H-part, slot]
                idxts = []
                xet = xet_pool.tile([P, NKH * 256], BF16, tag="xet")
                xet3 = xet[:].rearrange("p (kt c) -> p kt c", kt=NKH)
                for ct in range(nct):
                    r = rws[ct]
                    idxt = idx_pool.tile([P, 1], U32, tag="idxt")
                    nc.sync.dma_start(
                        idxt[0:r, :], table[offj + ct * P: offj + ct * P + r, :])
                    idxts.append(idxt)
                    xe_t = xe_pool.tile([P, H], BF16, tag="xe")
                    nc.gpsimd.indirect_dma_start(
                        xe_t[0:r, :], None, hs_pad[:],
                        bass.IndirectOffsetOnAxis(ap=idxt[0:r, :], axis=0))
                    for kt in range(NKH):
                        tr_ps = tr_ps_pool.tile([P, P], BF16, tag="trps")
                        nc.tensor.transpose(tr_ps[:, 0:r],
                                            xe_t[0:r, kt * P:(kt + 1) * P],
                                            id_t[0:r, 0:r])
                        nc.vector.tensor_copy(
                            xet3[:, kt, ct * P:ct * P + r], tr_ps[:, 0:r])

                # gate_up -> silu*up -> at  (w13 fp8e3 x xet bf16)
                at_t = at_pool.tile([P, NIT * 256], BF16, tag="at")
                at3 = at_t[:].rearrange("p (it c) -> p it c", it=NIT)
                w13_ts = []
                for wh in range(2):
                    w13_t = w13_pool.tile([P, NIT // 2 * NKH * 256], F8E3,
                                          tag="w13")
                    nc.sync.dma_start(
                        w13_t[:],
                        w13_pk[j, :, wh * (NIT // 2) * NKH * 256:
                               (wh + 1) * (NIT // 2) * NKH * 256])
                    w13_ts.append(w13_t)
                for mp in range(NIT):
                    w13_4 = w13_ts[mp // 4][:].rearrange(
                        "p (mp kt c) -> p mp kt c", mp=NIT // 2, kt=NKH)
                    hts = []
                    for half in range(2):
                        ht_ps = ht_ps_pool.tile([P, 256], F32, tag="htps")
                        for kt in range(NKH):
                            nc.tensor.matmul(
                                ht_ps[:, 0:capj],
                                w13_4[:, mp % 4, kt, half * P:(half + 1) * P],
                                xet3[:, kt, 0:capj],
                                start=(kt == 0), stop=(kt == NKH - 1))
                        hts.append(ht_ps)
                    sil = sil_pool.tile([P, 256], F32, tag="sil_e")
                    nc.scalar.activation(sil[:, 0:capj], hts[0][:, 0:capj],
                                         ACTF.Sigmoid, scale=1.0 / WS)
                    nc.vector.tensor_tensor(sil[:, 0:capj], sil[:, 0:capj],
                                            hts[0][:, 0:capj], ALU.mult)
                    nc.vector.tensor_tensor(at3[:, mp, 0:capj], sil[:, 0:capj],
                                            hts[1][:, 0:capj], ALU.mult)

                # down (at bf16 x w2 fp8e3), weight rows, scatter-add into acc
                ysrcs, wss = [], []
                for ct in range(nct):
                    ysrc_t = ys_pool.tile([P, H], BF16, tag="ysrc")
                    ysrcs.append(ysrc_t)
                    ws_t = idx_pool.tile([P, 1], F32, tag="wst")
                    nc.sync.dma_start(
                        ws_t[0:rws[ct], :],
                        wtbl[offj + ct * P: offj + ct * P + rws[ct], :])
                    wss.append(ws_t)
                for hc in range(4):
                    w2_t = w2_pool.tile([P, NIT * 512], F8E3, tag="w2")
                    w2_3 = w2_t[:].rearrange("p (it n) -> p it n", it=NIT)
                    nc.sync.dma_start(w2_t[:], w2_pk[j, hc])
                    for ct in range(nct):
                        r = rws[ct]
                        y_ps = y_ps_pool.tile([P, 512], F32, tag="yps")
                        for it in range(NIT):
                            nc.tensor.matmul(
                                y_ps[0:r, :],
                                at3[:, it, ct * P:ct * P + r],
                                w2_3[:, it, :],
                                start=(it == 0), stop=(it == NIT - 1))
                        nc.scalar.activation(
                            ysrcs[ct][0:r, hc * 512:(hc + 1) * 512],
                            y_ps[0:r, :], ACTF.Copy,
                            scale=wss[ct][0:r, 0:1])
                for ct in range(nct):
                    r = rws[ct]
                    nc.gpsimd.indirect_dma_start(
                        acc[:],
                        bass.IndirectOffsetOnAxis(ap=idxts[ct][0:r, :], axis=0),
                        ysrcs[ct][0:r, :], None,
                        compute_op=ALU.add)

        # ============ cross-core reduce-scatter (bf16)
        nc.gpsimd.collective_compute(
            "ReduceScatter", ALU.add,
            replica_groups=[list(range(NCORES))],
            ins=[acc[1:T + 1, :]],
            outs=[cc_out[:]],
        )
        nc.sync.dma_start(out_t[:], cc_out[:])

    nc.compile()
    return nc


# ---------------------------------------------------------------- host driver
def make_in_maps(inputs, experts_at, pattern, offs, SS):
    hs = np.ascontiguousarray(np.asarray(inputs["hidden_states"], np.float32))
    gate_w = np.asarray(inputs["gate_w"], np.float32)
    gate_bias = np.asarray(inputs["gate_bias"], np.float32)
    w13 = np.asarray(inputs["w13"], np.float32)
    w2 = np.asarray(inputs["w2"], np.float32)
    sgu = np.asarray(inputs["shared_gate_up"], np.float32)
    sd = np.asarray(inputs["shared_down"], np.float32)
    NJ = len(pattern)

    hs_pad = np.zeros((T + 1, H), bf16)
    hs_pad[1:] = hs.astype(bf16)
    hsT = np.ascontiguousarray(hs.T)
    hsT_bf = hsT.astype(bf16)
    gwT = np.ascontiguousarray(gate_w.T)
    bias_bc = np.ascontiguousarray(np.broadcast_to(gate_bias, (P, E)))
    u128 = (np.arange(P)[:, None] < np.arange(P)[None, :]).astype(np.float16)
    ones128 = np.ones((P, P), np.float16)
    onesc = np.ones((P, 1), np.float16)
    tokcol = (np.arange(NT)[None, :] * P + np.arange(P)[:, None] + 1
              ).astype(np.float16)
    ident = np.eye(P, dtype=bf16)
    eslot_row = np.zeros(SS, np.float16)
    ciota_row = np.zeros(SS, np.float16)
    for j in range(NJ):
        ciota_row[offs[j]:offs[j + 1]] = np.arange(pattern[j], dtype=np.float16)

    def to_f8(x):
        return np.clip(x * WS, -15.5, 15.5).astype(f8e3)

    sgu_g, sgu_u = sgu[:, :S], sgu[:, S:]

    in_maps = []
    for c in range(NCORES):
        erow = eslot_row.copy()
        w13p = np.zeros((NJ, P, NIT * NKH * 256), f8e3)
        w2p = np.zeros((NJ, 4, P, NIT * 512), f8e3)
        for j in range(NJ):
            e = experts_at[j][c]
            erow[offs[j]:offs[j + 1]] = e
            wg = w13[e, :, :I].reshape(H, NIT, P)
            wu = w13[e, :, I:].reshape(H, NIT, P)
            # [NIT(mp), H, 256] -> [NIT, NKH, 128, 256] -> [128, NIT, NKH, 256]
            wj = np.concatenate([wg, wu], axis=-1).transpose(1, 0, 2)
            wj = wj.reshape(NIT, NKH, P, 256).transpose(2, 0, 1, 3)
            w13p[j] = to_f8(wj.reshape(P, NIT * NKH * 256))
            # w2 [I, H] -> per hc: [NIT, 128, 512] -> [128, NIT*512]
            w2e = w2[e].reshape(NIT, P, 4, 512).transpose(2, 1, 0, 3)
            w2p[j] = to_f8(w2e.reshape(4, P, NIT * 512))
        sgu_c = np.concatenate([sgu_g[:, c * SL:(c + 1) * SL],
                                sgu_u[:, c * SL:(c + 1) * SL]], axis=1)
        in_maps.append({
            "hs_pad": hs_pad,
            "hsT": hsT,
            "hsT_bf": hsT_bf,
            "gwT": gwT,
            "bias_bc": bias_bc,
            "u128": u128,
            "ones128": ones128,
            "onesc": onesc,
            "tokcol": tokcol,
            "ident_b": ident,
            "eslot": np.ascontiguousarray(
                np.broadcast_to(erow, (P, SS))),
            "ciota": np.ascontiguousarray(
                np.broadcast_to(ciota_row, (P, SS))),
            "w13_pk": np.ascontiguousarray(w13p),
            "w2_pk": np.ascontiguousarray(w2p),
            "sgu_loc": np.ascontiguousarray(sgu_c).astype(bf16),
            "sd_loc": np.ascontiguousarray(sd[c * SL:(c + 1) * SL]).astype(bf16),
        })
    return in_maps


_PLAN_CACHE = {}


def plan_from_inputs(inputs):
    hs = np.asarray(inputs["hidden_states"], np.float32)
    gw = np.asarray(inputs["gate_w"], np.float32)
    gb = np.asarray(inputs["gate_bias"], np.float32)
    ids = _host_topk_ids(hs, gw, gb)
    cnt = np.bincount(ids.reshape(-1), minlength=E)
    return _plan(cnt)


def kernel(**inputs):
    pattern, experts_at, offs, SS = plan_from_inputs(inputs)
    key = tuple(pattern)
    if key not in _PLAN_CACHE:
        _PLAN_CACHE[key] = build_program(pattern, offs, SS)
    nc = _PLAN_CACHE[key]
    in_maps = make_in_maps(inputs, experts_at, pattern, offs, SS)
    res = bass_utils.run_bass_kernel_spmd(nc, in_maps,
                                          core_ids=list(range(NCORES)))
    out = np.concatenate([np.asarray(res.results[c]["out"]).astype(np.float32)
                          for c in range(NCORES)], axis=0)
    return out
